# revision 22
# baseline (speedup 1.0000x reference)
"""Dense MoE feed-forward for Trainium2, data-parallel over batch on 8 cores.

Reference computation (per token t):
    logits = x @ gate_w + gate_b                       # [E]
    gw     = softmax(top2-masked logits)               # [E]
    out    = sum_e gw[e] * (gelu(x @ w1[e] + b1[e]) @ w2[e] + b2[e])

Per-core device program (feature-major: [feature partitions, token free dim]):
  - gating matmuls in exact fp32, top-2 masked softmax on DVE/ACT
  - out accumulation entirely in 6 persistent PSUM banks:
      acc[j]  = b2^T @ gwT                (start of accumulation group)
      acc[j] += w2[e]^T @ (gelu(w1[e]^T @ xT + b1[e]) * gw_bcast[e])
    for all 7 experts; the gate weight is folded into the GELU output, so no
    per-expert combine pass exists.
  - main matmuls run as float32r (full PE rate at N=512, fp32-ish precision)
  - weights streamed from HBM in 2.3 MB quarter-expert DMAs, host-side
    pre-laid-out so every DMA is fully contiguous.

Host side shards x over cores (1 batch row each), replicates weights,
pre-transposes x -> xT and un-transposes the feature-major out.
"""

import numpy as np

B, S, H, I, E = 8, 512, 768, 3072, 7
P = 128
KC = H // P  # 6 h-chunks (contraction for mm1, output chunks for mm2)
IC = I // P  # 24 i-chunks
NQ = 4  # weight streaming quarters per expert
OQ = IC // NQ  # 6 i-chunks per quarter
T = S  # tokens per core
NT = T // P  # 4 token tiles (gating)

F32 = None  # filled lazily (mybir import inside functions)

_CACHE = {}


def _build_program():
    import concourse.bass as bass
    import concourse.mybir as mybir
    import concourse.tile as tile
    from concourse import bacc
    from concourse.masks import make_identity

    f32 = mybir.dt.float32
    # float32r: fp32 bits consumed by the PE in "replicated" mode — full
    # bf16-rate at N>=512 with ~2^-13 effective precision (HW-probed).
    # Gating stays in exact fp32 (top-2 selection is flip-sensitive).
    f32r = mybir.dt.float32r
    AF = mybir.ActivationFunctionType
    OP = mybir.AluOpType
    AX = mybir.AxisListType

    nc = bacc.Bacc("TRN2", target_bir_lowering=False, debug=False, num_devices=8)

    # All DMA-touched tensors are flat 2D [partitions, contiguous free] so each
    # partition transfers as one maximal packet (nested free dims were observed
    # to emit one packet per innermost run, e.g. 28B packets for gate_w).
    xt_d = nc.dram_tensor("xt", [P, KC * T], f32, kind="ExternalInput").ap()
    xtr_d = nc.dram_tensor("xt_r", [P, KC * T], f32r, kind="ExternalInput").ap()
    gate_w_d = nc.dram_tensor("gate_w_r", [P, KC * E], f32, kind="ExternalInput").ap()
    gate_b_d = nc.dram_tensor("gate_b_r", [1, E], f32, kind="ExternalInput").ap()
    w1_d = nc.dram_tensor(
        "w1_r", [E, NQ, P, KC * OQ * P], f32r, kind="ExternalInput"
    ).ap()
    b1_d = nc.dram_tensor("b1_r", [P, E * IC], f32, kind="ExternalInput").ap()
    w2_d = nc.dram_tensor("w2_r", [E, NQ, P, OQ * H], f32r, kind="ExternalInput").ap()
    b2_d = nc.dram_tensor("b2_r", [E, H], f32, kind="ExternalInput").ap()
    sel_d = nc.dram_tensor("sel_r", [E, E * P], f32, kind="ExternalInput").ap()
    out_d = nc.dram_tensor("out_fm", [KC, P, T], f32, kind="ExternalOutput").ap()
    gw_d = nc.dram_tensor("gw_out", [T, E], f32, kind="ExternalOutput").ap()

    with tile.TileContext(nc) as tc:
        with (
            tc.tile_pool(name="const", bufs=1) as const,
            tc.tile_pool(name="w1p", bufs=4) as w1p,
            tc.tile_pool(name="w2p", bufs=3) as w2p,
            tc.tile_pool(name="gwbp", bufs=2) as gwbp,
            tc.tile_pool(name="hp", bufs=2) as hp,
            tc.tile_pool(name="hsp", bufs=2) as hsp,
            tc.tile_pool(name="gat", bufs=2) as gat,
            tc.tile_pool(name="outp", bufs=2) as outp,
            tc.tile_pool(name="pacc", bufs=1, space="PSUM") as pacc,
            tc.tile_pool(name="pwork", bufs=2, space="PSUM") as pwork,
        ):
            # ---- constants / small inputs ----
            ident = const.tile([P, P], f32)
            make_identity(nc, ident[:])
            ones_row = const.tile([1, P], f32)
            nc.gpsimd.memset(ones_row[:], 1.0)
            # DMA ring usage (HWDGE FIFOs execute in emission order per engine):
            #   scalar ring: gating consts (sel, gate_w, gate_b), gw outputs
            #   sync ring:   b2, first w1 quarter, xt, then the w1 stream, out
            #   gpsimd ring: xtr, b1, then the w2 quarter stream
            # This keeps the first mm1 gated only on [w1q(0,0) | xtr] and the
            # gating matmuls gated only on [xt | gate consts].
            # sel row block e is an [E, P] stationary that replicates gwT row
            # e across all 128 output partitions.
            sel = const.tile([E, E * P], f32)
            nc.scalar.dma_start(sel[:], sel_d)
            gw_sb = const.tile([P, KC * E], f32)
            nc.scalar.dma_start(gw_sb[:], gate_w_d)
            gb_sb = const.tile([1, E], f32)
            nc.scalar.dma_start(gb_sb[:], gate_b_d)
            b2_sb = const.tile([E, H], f32)
            nc.sync.dma_start(b2_sb[:], b2_d)

            w1_tiles = {}
            w2_tiles = {}

            def load_w1q(e, q):
                w1q = w1p.tile([P, KC * OQ * P], f32r, tag="w1q", name=f"w1q_{e}_{q}")
                nc.sync.dma_start(w1q[:], w1_d[e, q])
                w1_tiles[(e, q)] = w1q

            def load_w2q(e, q):
                w2q = w2p.tile([P, OQ * H], f32r, tag="w2q", name=f"w2q_{e}_{q}")
                nc.gpsimd.dma_start(w2q[:], w2_d[e, q])
                w2_tiles[(e, q)] = w2q

            load_w1q(0, 0)
            xt = const.tile([P, KC * T], f32)
            nc.sync.dma_start(xt[:], xt_d)
            xtr = const.tile([P, KC * T], f32r)
            nc.gpsimd.dma_start(xtr[:], xtr_d)
            b1_sb = const.tile([P, E * IC], f32)
            nc.gpsimd.dma_start(b1_sb[:], b1_d)
            load_w2q(0, 0)
            gwT = const.tile([E, T], f32)

            # ---- gating: logits -> top-2 masked softmax (token-major) ----
            for ti in range(NT):
                tsl = slice(ti * P, (ti + 1) * P)
                lg_ps = pwork.tile([P, E], f32, tag="pw")
                for k in range(KC):
                    nc.tensor.matmul(
                        lg_ps[:],
                        xt[:, k * T + ti * P : k * T + (ti + 1) * P],
                        gw_sb[:, k * E : (k + 1) * E],
                        start=(k == 0),
                        stop=False,
                    )
                nc.tensor.matmul(lg_ps[:], ones_row[:], gb_sb[:], start=False, stop=True)

                lg = gat.tile([P, E], f32, tag="lg")
                nc.vector.tensor_copy(lg[:], lg_ps[:])
                m1 = gat.tile([P, 1], f32, tag="m1")
                nc.vector.reduce_max(m1[:], lg[:], axis=AX.X)
                nm1 = gat.tile([P, 1], f32, tag="nm1")
                nc.vector.tensor_scalar_mul(nm1[:], m1[:], -1.0)
                eq = gat.tile([P, E], f32, tag="eq")
                nc.vector.tensor_scalar(eq[:], lg[:], m1[:], None, op0=OP.is_equal)
                masked = gat.tile([P, E], f32, tag="masked")
                nc.vector.scalar_tensor_tensor(
                    masked[:], eq[:], -1e30, lg[:], op0=OP.mult, op1=OP.add
                )
                m2 = gat.tile([P, 1], f32, tag="m2")
                nc.vector.reduce_max(m2[:], masked[:], axis=AX.X)
                ge = gat.tile([P, E], f32, tag="ge")
                nc.vector.tensor_scalar(ge[:], lg[:], m2[:], None, op0=OP.is_ge)
                ex = gat.tile([P, E], f32, tag="ex")
                nc.scalar.activation(ex[:], lg[:], AF.Exp, bias=nm1[:])
                ek = gat.tile([P, E], f32, tag="ek")
                nc.vector.tensor_tensor(ek[:], ex[:], ge[:], op=OP.mult)
                z = gat.tile([P, 1], f32, tag="z")
                nc.vector.reduce_sum(z[:], ek[:], axis=AX.X)
                rz = gat.tile([P, 1], f32, tag="rz")
                nc.vector.reciprocal(rz[:], z[:])
                gwt = gat.tile([P, E], f32, tag="gwt")
                nc.vector.tensor_scalar_mul(gwt[:], ek[:], rz[:])
                nc.scalar.dma_start(gw_d[tsl, :], gwt[:])
                # transpose [P, E] -> [E, P] and place into gwT
                tp = pwork.tile([E, P], f32, tag="pw")
                nc.tensor.matmul(tp[:], gwt[:], ident[:], is_transpose=True)
                nc.vector.tensor_copy(gwT[:, tsl], tp[:])

            # ---- init out accumulators with sum_e gw[e]*b2[e] (exact fp32) ----
            acc = []
            for j in range(KC):
                a = pacc.tile([P, T], f32, tag=f"acc{j}")
                nc.tensor.matmul(
                    a[:],
                    b2_sb[:, j * P : (j + 1) * P],
                    gwT[:],
                    start=True,
                    stop=False,
                )
                acc.append(a)

            # ---- expert loop ----
            for e in range(E):
                # broadcast gw row -> [P, T] (exact fp32: 1.0 * gw)
                bc_ps = pwork.tile([P, T], f32, tag="pw")
                nc.tensor.matmul(
                    bc_ps[:], sel[:, e * P : (e + 1) * P], gwT[:], start=True, stop=True
                )
                gwb = gwbp.tile([P, T], f32, tag="gwb")
                nc.vector.tensor_copy(gwb[:], bc_ps[:])

                for q in range(NQ):
                    if (e, q) not in w1_tiles:
                        load_w1q(e, q)
                    if (e, q) not in w2_tiles:
                        load_w2q(e, q)
                    w1q = w1_tiles.pop((e, q))
                    w2q = w2_tiles.pop((e, q))
                    for c in range(OQ):
                        i = q * OQ + c
                        hp_ps = pwork.tile([P, T], f32, tag="pw")
                        for k in range(KC):
                            nc.tensor.matmul(
                                hp_ps[:],
                                w1q[:, k * OQ * P + c * P : k * OQ * P + (c + 1) * P],
                                xtr[:, k * T : (k + 1) * T],
                                start=(k == 0),
                                stop=(k == KC - 1),
                            )
                        h_sb = hp.tile([P, T], f32, tag="h")
                        nc.scalar.activation(
                            h_sb[:],
                            hp_ps[:],
                            AF.Gelu,
                            bias=b1_sb[:, e * IC + i : e * IC + i + 1],
                        )
                        hs = hsp.tile([P, T], f32r, tag="hs")
                        nc.vector.tensor_tensor(hs[:], h_sb[:], gwb[:], op=OP.mult)
                        last = e == E - 1 and i == IC - 1
                        for j in range(KC):
                            nc.tensor.matmul(
                                acc[j][:],
                                w2q[:, c * H + j * P : c * H + (j + 1) * P],
                                hs[:],
                                start=False,
                                stop=last,
                            )

            # ---- drain accumulators ----
            for j in range(KC):
                osb = outp.tile([P, T], f32, tag="osb")
                nc.scalar.copy(osb[:], acc[j][:])
                nc.sync.dma_start(out_d[j], osb[:])

    nc.compile()
    return nc


def _get_program():
    if "nc" not in _CACHE:
        _CACHE["nc"] = _build_program()
    return _CACHE["nc"]


def _prep_shared(gate_w, gate_b, w1, b1, w2, b2):
    f = np.float32
    gate_w_r = np.ascontiguousarray(
        gate_w.reshape(KC, P, E).transpose(1, 0, 2), dtype=f
    ).reshape(P, KC * E)
    gate_b_r = np.ascontiguousarray(gate_b.reshape(1, E), dtype=f)
    w1_r = np.ascontiguousarray(
        w1.reshape(E, KC, P, NQ, OQ * P).transpose(0, 3, 2, 1, 4), dtype=f
    ).reshape(E, NQ, P, KC * OQ * P)
    b1_r = np.ascontiguousarray(b1.reshape(E, IC, P).transpose(2, 0, 1), dtype=f).reshape(
        P, E * IC
    )
    w2_r = np.ascontiguousarray(
        w2.reshape(E, NQ, OQ, P, H).transpose(0, 1, 3, 2, 4), dtype=f
    ).reshape(E, NQ, P, OQ * H)
    b2_r = np.ascontiguousarray(b2, dtype=f)
    return {
        "gate_w_r": gate_w_r,
        "gate_b_r": gate_b_r,
        "w1_r": w1_r,
        "b1_r": b1_r,
        "w2_r": w2_r,
        "b2_r": b2_r,
        "sel_r": np.ascontiguousarray(
            np.broadcast_to(np.eye(E, dtype=f)[:, :, None], (E, E, P))
        ).reshape(E, E * P),
    }


def _make_in_maps(x, gate_w, gate_b, w1, b1, w2, b2):
    shared = _prep_shared(gate_w, gate_b, w1, b1, w2, b2)
    in_maps = []
    for c in range(B):
        xt = np.ascontiguousarray(
            np.asarray(x[c], dtype=np.float32).T.reshape(KC, P, T).transpose(1, 0, 2)
        ).reshape(P, KC * T)
        m = dict(shared)
        m["xt"] = xt
        m["xt_r"] = xt
        in_maps.append(m)
    return in_maps


def run_on_hw(in_maps, trace=False):
    from concourse.bass_interp import get_hw_module
    from concourse.bass_utils import run_bass_kernel_spmd

    nc = _get_program()
    old_m = nc.m
    nc.m = get_hw_module(nc.m)
    try:
        return run_bass_kernel_spmd(
            nc, in_maps, core_ids=list(range(B)), trace=trace
        )
    finally:
        nc.m = old_m


def _assemble(results):
    out = np.empty((B, S, H), dtype=np.float32)
    gws = np.empty((B, S, E), dtype=np.float32)
    for c in range(B):
        out_fm = results[c]["out_fm"]  # [KC, P, T]
        out[c] = out_fm.transpose(2, 0, 1).reshape(T, H)
        gws[c] = results[c]["gw_out"]
    return out, gws


def kernel(x, gate_w, gate_b, w1, b1, w2, b2):
    in_maps = _make_in_maps(x, gate_w, gate_b, w1, b1, w2, b2)
    res = run_on_hw(in_maps, trace=False)
    return _assemble(res.results)


# revision 27
# speedup vs baseline: 1.0231x; 1.0231x over previous
"""Dense MoE feed-forward for Trainium2, data-parallel over batch on 8 cores.

Reference computation (per token t):
    logits = x @ gate_w + gate_b                       # [E]
    gw     = softmax(top2-masked logits)               # [E]
    out    = sum_e gw[e] * (gelu(x @ w1[e] + b1[e]) @ w2[e] + b2[e])

Per-core device program (feature-major: [feature partitions, token free dim]):
  - gating matmuls in exact fp32, top-2 masked softmax on DVE/ACT
  - out accumulation entirely in 6 persistent PSUM banks:
      acc[j]  = b2^T @ gwT                (start of accumulation group)
      acc[j] += w2[e]^T @ (gelu(w1[e]^T @ xT + b1[e]) * gw_bcast[e])
    for all 7 experts; the gate weight is folded into the GELU output, so no
    per-expert combine pass exists.
  - main matmuls run as float32r (full PE rate at N=512, fp32-ish precision)
  - weights streamed from HBM in 2.3 MB quarter-expert DMAs, host-side
    pre-laid-out so every DMA is fully contiguous.

Host side shards x over cores (1 batch row each), replicates weights,
pre-transposes x -> xT and un-transposes the feature-major out.
"""

import numpy as np

B, S, H, I, E = 8, 512, 768, 3072, 7
P = 128
KC = H // P  # 6 h-chunks (contraction for mm1, output chunks for mm2)
IC = I // P  # 24 i-chunks
NQ = 4  # weight streaming quarters per expert
OQ = IC // NQ  # 6 i-chunks per quarter
T = S  # tokens per core
NT = T // P  # 4 token tiles (gating)

F32 = None  # filled lazily (mybir import inside functions)

_CACHE = {}


def _build_program():
    import concourse.bass as bass
    import concourse.mybir as mybir
    import concourse.tile as tile
    from concourse import bacc
    from concourse.masks import make_identity

    f32 = mybir.dt.float32
    # float32r: fp32 bits consumed by the PE in "replicated" mode — full
    # bf16-rate at N>=512 with ~2^-13 effective precision (HW-probed).
    # Gating stays in exact fp32 (top-2 selection is flip-sensitive).
    f32r = mybir.dt.float32r
    AF = mybir.ActivationFunctionType
    OP = mybir.AluOpType
    AX = mybir.AxisListType

    nc = bacc.Bacc("TRN2", target_bir_lowering=False, debug=False, num_devices=8)

    # All DMA-touched tensors are flat 2D [partitions, contiguous free] so each
    # partition transfers as one maximal packet (nested free dims were observed
    # to emit one packet per innermost run, e.g. 28B packets for gate_w).
    # gate_w and b1 ship in 7-partition natural layouts (7 big packets instead
    # of 128 tiny ones — small packets pace at ~230ns each) and are transposed
    # into per-partition layouts on device.
    xt_d = nc.dram_tensor("xt", [P, KC * T], f32, kind="ExternalInput").ap()
    xtr_d = nc.dram_tensor("xt_r", [P, KC * T], f32r, kind="ExternalInput").ap()
    gate_w_d = nc.dram_tensor("gate_w_r", [E, H], f32, kind="ExternalInput").ap()
    gate_b_d = nc.dram_tensor("gate_b_r", [1, E], f32, kind="ExternalInput").ap()
    w1_d = nc.dram_tensor(
        "w1_r", [E, NQ, P, KC * OQ * P], f32r, kind="ExternalInput"
    ).ap()
    b1_d = nc.dram_tensor("b1_r", [E, I], f32, kind="ExternalInput").ap()
    w2_d = nc.dram_tensor("w2_r", [E, NQ, P, OQ * H], f32r, kind="ExternalInput").ap()
    b2_d = nc.dram_tensor("b2_r", [E, H], f32, kind="ExternalInput").ap()
    sel_d = nc.dram_tensor("sel_r", [E, E * P], f32, kind="ExternalInput").ap()
    out_d = nc.dram_tensor("out_fm", [KC, P, T], f32, kind="ExternalOutput").ap()
    gw_d = nc.dram_tensor("gw_out", [T, E], f32, kind="ExternalOutput").ap()

    with tile.TileContext(nc) as tc:
        with (
            tc.tile_pool(name="const", bufs=1) as const,
            tc.tile_pool(name="w1p", bufs=4) as w1p,
            tc.tile_pool(name="w2p", bufs=3) as w2p,
            tc.tile_pool(name="gwbp", bufs=2) as gwbp,
            tc.tile_pool(name="hp", bufs=2) as hp,
            tc.tile_pool(name="hsp", bufs=2) as hsp,
            tc.tile_pool(name="gat", bufs=2) as gat,
            tc.tile_pool(name="outp", bufs=2) as outp,
            tc.tile_pool(name="pacc", bufs=1, space="PSUM") as pacc,
            tc.tile_pool(name="pwork", bufs=2, space="PSUM") as pwork,
        ):
            # ---- constants / small inputs ----
            ident = const.tile([P, P], f32)
            make_identity(nc, ident[:])
            ones_row = const.tile([1, P], f32)
            nc.gpsimd.memset(ones_row[:], 1.0)
            # DMA ring usage (HWDGE FIFOs execute in emission order per engine):
            #   scalar ring: gating consts (gate_w, gate_b, sel), xt, gw outs
            #   sync ring:   b2, first w1 quarter, then the w1 stream, out
            #   gpsimd ring: b1, xtr, then the w2 quarter stream
            # This keeps the first mm1 gated only on [w1q(0,0) | xtr] and the
            # gating matmuls gated only on [xt | gate consts].
            # sel row block e is an [E, P] stationary that replicates gwT row
            # e across all 128 output partitions.
            gwn_sb = const.tile([E, H], f32)
            nc.scalar.dma_start(gwn_sb[:], gate_w_d)
            gb_sb = const.tile([1, E], f32)
            nc.scalar.dma_start(gb_sb[:], gate_b_d)
            sel = const.tile([E, E * P], f32)
            nc.scalar.dma_start(sel[:], sel_d)
            xt = const.tile([P, KC * T], f32)
            nc.scalar.dma_start(xt[:], xt_d)
            b2_sb = const.tile([E, H], f32)
            nc.sync.dma_start(b2_sb[:], b2_d)

            w1_tiles = {}
            w2_tiles = {}

            def load_w1q(e, q):
                w1q = w1p.tile([P, KC * OQ * P], f32r, tag="w1q", name=f"w1q_{e}_{q}")
                nc.sync.dma_start(w1q[:], w1_d[e, q])
                w1_tiles[(e, q)] = w1q

            def load_w2q(e, q):
                w2q = w2p.tile([P, OQ * H], f32r, tag="w2q", name=f"w2q_{e}_{q}")
                nc.gpsimd.dma_start(w2q[:], w2_d[e, q])
                w2_tiles[(e, q)] = w2q

            load_w1q(0, 0)
            b1n_sb = const.tile([E, I], f32)
            nc.gpsimd.dma_start(b1n_sb[:], b1_d)
            xtr = const.tile([P, KC * T], f32r)
            nc.gpsimd.dma_start(xtr[:], xtr_d)
            load_w2q(0, 0)
            gwT = const.tile([E, T], f32)

            # ---- derive per-partition layouts of gate_w and b1 via PE ----
            # transpose([7, 128] block) -> psum [128, 7] -> SBUF
            ident7 = ident[:E, :E]
            gw_sb = const.tile([P, KC * E], f32)
            for k in range(KC):
                tpg = pwork.tile([P, E], f32, tag="pw")
                nc.tensor.matmul(
                    tpg[:], gwn_sb[:, k * P : (k + 1) * P], ident7, is_transpose=True
                )
                nc.vector.tensor_copy(gw_sb[:, k * E : (k + 1) * E], tpg[:])
            b1_sb = const.tile([P, IC * E], f32)
            for i in range(IC):
                tpb = pwork.tile([P, E], f32, tag="pw")
                nc.tensor.matmul(
                    tpb[:], b1n_sb[:, i * P : (i + 1) * P], ident7, is_transpose=True
                )
                nc.vector.tensor_copy(b1_sb[:, i * E : (i + 1) * E], tpb[:])

            # ---- gating: logits -> top-2 masked softmax (token-major) ----
            gwt_tiles = []
            for ti in range(NT):
                tsl = slice(ti * P, (ti + 1) * P)
                lg_ps = pwork.tile([P, E], f32, tag="pw")
                for k in range(KC):
                    nc.tensor.matmul(
                        lg_ps[:],
                        xt[:, k * T + ti * P : k * T + (ti + 1) * P],
                        gw_sb[:, k * E : (k + 1) * E],
                        start=(k == 0),
                        stop=False,
                    )
                nc.tensor.matmul(lg_ps[:], ones_row[:], gb_sb[:], start=False, stop=True)

                lg = gat.tile([P, E], f32, tag="lg")
                nc.vector.tensor_copy(lg[:], lg_ps[:])
                m1 = gat.tile([P, 1], f32, tag="m1")
                nc.vector.reduce_max(m1[:], lg[:], axis=AX.X)
                nm1 = gat.tile([P, 1], f32, tag="nm1")
                nc.vector.tensor_scalar_mul(nm1[:], m1[:], -1.0)
                eq = gat.tile([P, E], f32, tag="eq")
                nc.vector.tensor_scalar(eq[:], lg[:], m1[:], None, op0=OP.is_equal)
                masked = gat.tile([P, E], f32, tag="masked")
                nc.vector.scalar_tensor_tensor(
                    masked[:], eq[:], -1e30, lg[:], op0=OP.mult, op1=OP.add
                )
                m2 = gat.tile([P, 1], f32, tag="m2")
                nc.vector.reduce_max(m2[:], masked[:], axis=AX.X)
                ge = gat.tile([P, E], f32, tag="ge")
                nc.vector.tensor_scalar(ge[:], lg[:], m2[:], None, op0=OP.is_ge)
                ex = gat.tile([P, E], f32, tag="ex")
                nc.scalar.activation(ex[:], lg[:], AF.Exp, bias=nm1[:])
                ek = gat.tile([P, E], f32, tag="ek")
                nc.vector.tensor_tensor(ek[:], ex[:], ge[:], op=OP.mult)
                z = gat.tile([P, 1], f32, tag="z")
                nc.vector.reduce_sum(z[:], ek[:], axis=AX.X)
                rz = gat.tile([P, 1], f32, tag="rz")
                nc.vector.reciprocal(rz[:], z[:])
                gwt = gat.tile([P, E], f32, tag="gwt", bufs=NT)
                nc.vector.tensor_scalar_mul(gwt[:], ek[:], rz[:])
                gwt_tiles.append((tsl, gwt))
                # transpose [P, E] -> [E, P] and place into gwT
                tp = pwork.tile([E, P], f32, tag="pw")
                nc.tensor.matmul(tp[:], gwt[:], ident[:], is_transpose=True)
                nc.vector.tensor_copy(gwT[:, tsl], tp[:])
            for tsl, gwt in gwt_tiles:
                nc.scalar.dma_start(gw_d[tsl, :], gwt[:])

            # ---- init out accumulators with sum_e gw[e]*b2[e] (exact fp32) ----
            acc = []
            for j in range(KC):
                a = pacc.tile([P, T], f32, tag=f"acc{j}")
                nc.tensor.matmul(
                    a[:],
                    b2_sb[:, j * P : (j + 1) * P],
                    gwT[:],
                    start=True,
                    stop=False,
                )
                acc.append(a)

            # ---- expert loop ----
            for e in range(E):
                # broadcast gw row -> [P, T] (exact fp32: 1.0 * gw)
                bc_ps = pwork.tile([P, T], f32, tag="pw")
                nc.tensor.matmul(
                    bc_ps[:], sel[:, e * P : (e + 1) * P], gwT[:], start=True, stop=True
                )
                gwb = gwbp.tile([P, T], f32, tag="gwb")
                nc.vector.tensor_copy(gwb[:], bc_ps[:])

                for q in range(NQ):
                    if (e, q) not in w1_tiles:
                        load_w1q(e, q)
                    if (e, q) not in w2_tiles:
                        load_w2q(e, q)
                    w1q = w1_tiles.pop((e, q))
                    w2q = w2_tiles.pop((e, q))
                    for c in range(OQ):
                        i = q * OQ + c
                        hp_ps = pwork.tile([P, T], f32, tag="pw")
                        for k in range(KC):
                            nc.tensor.matmul(
                                hp_ps[:],
                                w1q[:, k * OQ * P + c * P : k * OQ * P + (c + 1) * P],
                                xtr[:, k * T : (k + 1) * T],
                                start=(k == 0),
                                stop=(k == KC - 1),
                            )
                        h_sb = hp.tile([P, T], f32, tag="h")
                        nc.scalar.activation(
                            h_sb[:],
                            hp_ps[:],
                            AF.Gelu,
                            bias=b1_sb[:, i * E + e : i * E + e + 1],
                        )
                        hs = hsp.tile([P, T], f32r, tag="hs")
                        nc.vector.tensor_tensor(hs[:], h_sb[:], gwb[:], op=OP.mult)
                        last = e == E - 1 and i == IC - 1
                        for j in range(KC):
                            nc.tensor.matmul(
                                acc[j][:],
                                w2q[:, c * H + j * P : c * H + (j + 1) * P],
                                hs[:],
                                start=False,
                                stop=last,
                            )

            # ---- drain accumulators ----
            for j in range(KC):
                osb = outp.tile([P, T], f32, tag="osb")
                nc.scalar.copy(osb[:], acc[j][:])
                nc.sync.dma_start(out_d[j], osb[:])

    nc.compile()
    return nc


def _get_program():
    if "nc" not in _CACHE:
        _CACHE["nc"] = _build_program()
    return _CACHE["nc"]


def _prep_shared(gate_w, gate_b, w1, b1, w2, b2):
    f = np.float32
    gate_w_r = np.ascontiguousarray(gate_w.T, dtype=f)
    gate_b_r = np.ascontiguousarray(gate_b.reshape(1, E), dtype=f)
    w1_r = np.ascontiguousarray(
        w1.reshape(E, KC, P, NQ, OQ * P).transpose(0, 3, 2, 1, 4), dtype=f
    ).reshape(E, NQ, P, KC * OQ * P)
    b1_r = np.ascontiguousarray(b1, dtype=f)
    w2_r = np.ascontiguousarray(
        w2.reshape(E, NQ, OQ, P, H).transpose(0, 1, 3, 2, 4), dtype=f
    ).reshape(E, NQ, P, OQ * H)
    b2_r = np.ascontiguousarray(b2, dtype=f)
    return {
        "gate_w_r": gate_w_r,
        "gate_b_r": gate_b_r,
        "w1_r": w1_r,
        "b1_r": b1_r,
        "w2_r": w2_r,
        "b2_r": b2_r,
        "sel_r": np.ascontiguousarray(
            np.broadcast_to(np.eye(E, dtype=f)[:, :, None], (E, E, P))
        ).reshape(E, E * P),
    }


def _make_in_maps(x, gate_w, gate_b, w1, b1, w2, b2):
    shared = _prep_shared(gate_w, gate_b, w1, b1, w2, b2)
    in_maps = []
    for c in range(B):
        xt = np.ascontiguousarray(
            np.asarray(x[c], dtype=np.float32).T.reshape(KC, P, T).transpose(1, 0, 2)
        ).reshape(P, KC * T)
        m = dict(shared)
        m["xt"] = xt
        m["xt_r"] = xt
        in_maps.append(m)
    return in_maps


def run_on_hw(in_maps, trace=False):
    from concourse.bass_interp import get_hw_module
    from concourse.bass_utils import run_bass_kernel_spmd

    nc = _get_program()
    old_m = nc.m
    nc.m = get_hw_module(nc.m)
    try:
        return run_bass_kernel_spmd(
            nc, in_maps, core_ids=list(range(B)), trace=trace
        )
    finally:
        nc.m = old_m


def _assemble(results):
    out = np.empty((B, S, H), dtype=np.float32)
    gws = np.empty((B, S, E), dtype=np.float32)
    for c in range(B):
        out_fm = results[c]["out_fm"]  # [KC, P, T]
        out[c] = out_fm.transpose(2, 0, 1).reshape(T, H)
        gws[c] = results[c]["gw_out"]
    return out, gws


def kernel(x, gate_w, gate_b, w1, b1, w2, b2):
    in_maps = _make_in_maps(x, gate_w, gate_b, w1, b1, w2, b2)
    res = run_on_hw(in_maps, trace=False)
    return _assemble(res.results)


# revision 28
# speedup vs baseline: 1.0711x; 1.0469x over previous
"""Dense MoE feed-forward for Trainium2, data-parallel over batch on 8 cores.

Reference computation (per token t):
    logits = x @ gate_w + gate_b                       # [E]
    gw     = softmax(top2-masked logits)               # [E]
    out    = sum_e gw[e] * (gelu(x @ w1[e] + b1[e]) @ w2[e] + b2[e])

Per-core device program (feature-major: [feature partitions, token free dim]):
  - gating matmuls in exact fp32, top-2 masked softmax on DVE/ACT
  - out accumulation entirely in 6 persistent PSUM banks:
      acc[j]  = b2^T @ gwT                (start of accumulation group)
      acc[j] += w2[e]^T @ (gelu(w1[e]^T @ xT + b1[e]) * gw_bcast[e])
    for all 7 experts; the gate weight is folded into the GELU output, so no
    per-expert combine pass exists.
  - main matmuls run as float32r (full PE rate at N=512, fp32-ish precision)
  - weights streamed from HBM in 2.3 MB quarter-expert DMAs, host-side
    pre-laid-out so every DMA is fully contiguous.

Host side shards x over cores (1 batch row each), replicates weights,
pre-transposes x -> xT and un-transposes the feature-major out.
"""

import numpy as np

B, S, H, I, E = 8, 512, 768, 3072, 7
P = 128
KC = H // P  # 6 h-chunks (contraction for mm1, output chunks for mm2)
IC = I // P  # 24 i-chunks
NQ = 4  # weight streaming quarters per expert
OQ = IC // NQ  # 6 i-chunks per quarter
T = S  # tokens per core
NT = T // P  # 4 token tiles (gating)

F32 = None  # filled lazily (mybir import inside functions)

_CACHE = {}


def _build_program():
    import concourse.bass as bass
    import concourse.mybir as mybir
    import concourse.tile as tile
    from concourse import bacc
    from concourse.masks import make_identity

    f32 = mybir.dt.float32
    # float32r: fp32 bits consumed by the PE in "replicated" mode — full
    # bf16-rate at N>=512 with ~2^-13 effective precision (HW-probed).
    # Gating stays in exact fp32 (top-2 selection is flip-sensitive).
    f32r = mybir.dt.float32r
    AF = mybir.ActivationFunctionType
    OP = mybir.AluOpType
    AX = mybir.AxisListType

    nc = bacc.Bacc("TRN2", target_bir_lowering=False, debug=False, num_devices=8)

    # All DMA-touched tensors are flat 2D [partitions, contiguous free] so each
    # partition transfers as one maximal packet (nested free dims were observed
    # to emit one packet per innermost run, e.g. 28B packets for gate_w).
    # gate_w and b1 ship in 7-partition natural layouts (7 big packets instead
    # of 128 tiny ones — small packets pace at ~230ns each) and are transposed
    # into per-partition layouts on device.
    xt_d = nc.dram_tensor("xt", [P, KC * T], f32, kind="ExternalInput").ap()
    xtr_d = nc.dram_tensor("xt_r", [P, KC * T], f32r, kind="ExternalInput").ap()
    gate_w_d = nc.dram_tensor("gate_w_r", [E, H], f32, kind="ExternalInput").ap()
    gate_b_d = nc.dram_tensor("gate_b_r", [1, E], f32, kind="ExternalInput").ap()
    w1_d = nc.dram_tensor(
        "w1_r", [E, NQ, P, KC * OQ * P], f32r, kind="ExternalInput"
    ).ap()
    b1_d = nc.dram_tensor("b1_r", [E, I], f32, kind="ExternalInput").ap()
    w2_d = nc.dram_tensor("w2_r", [E, NQ, P, OQ * H], f32r, kind="ExternalInput").ap()
    b2_d = nc.dram_tensor("b2_r", [E, H], f32, kind="ExternalInput").ap()
    sel_d = nc.dram_tensor("sel_r", [E, E * P], f32, kind="ExternalInput").ap()
    out_d = nc.dram_tensor("out_fm", [KC, P, T], f32, kind="ExternalOutput").ap()
    gw_d = nc.dram_tensor("gw_out", [T, E], f32, kind="ExternalOutput").ap()

    with tile.TileContext(nc) as tc:
        with (
            tc.tile_pool(name="const", bufs=1) as const,
            tc.tile_pool(name="w1p", bufs=4) as w1p,
            tc.tile_pool(name="w2p", bufs=3) as w2p,
            tc.tile_pool(name="gwbp", bufs=2) as gwbp,
            tc.tile_pool(name="hp", bufs=2) as hp,
            tc.tile_pool(name="hsp", bufs=2) as hsp,
            tc.tile_pool(name="gat", bufs=2) as gat,
            tc.tile_pool(name="outp", bufs=2) as outp,
            tc.tile_pool(name="pacc", bufs=1, space="PSUM") as pacc,
            tc.tile_pool(name="pwork", bufs=2, space="PSUM") as pwork,
        ):
            # ---- constants / small inputs ----
            ident = const.tile([P, P], f32)
            make_identity(nc, ident[:])
            ones_row = const.tile([1, P], f32)
            nc.gpsimd.memset(ones_row[:], 1.0)
            # DMA ring usage (HWDGE FIFOs execute in emission order per engine):
            #   scalar ring: gating consts (gate_w, gate_b, sel), xt, gw outs
            #   sync ring:   b2, first w1 quarter, then the w1 stream, out
            #   gpsimd ring: b1, xtr, then the w2 quarter stream
            # This keeps the first mm1 gated only on [w1q(0,0) | xtr] and the
            # gating matmuls gated only on [xt | gate consts].
            # sel row block e is an [E, P] stationary that replicates gwT row
            # e across all 128 output partitions.
            gwn_sb = const.tile([E, H], f32)
            nc.scalar.dma_start(gwn_sb[:], gate_w_d)
            gb_sb = const.tile([1, E], f32)
            nc.scalar.dma_start(gb_sb[:], gate_b_d)
            sel = const.tile([E, E * P], f32)
            nc.scalar.dma_start(sel[:], sel_d)
            b2_sb = const.tile([E, H], f32)
            nc.sync.dma_start(b2_sb[:], b2_d)
            # xt on the sync ring: the scalar/ACT ring is low-bandwidth (its
            # 1.5 MB took ~35us there, stalling the gating matmuls).
            xt = const.tile([P, KC * T], f32)
            nc.sync.dma_start(xt[:], xt_d)

            w1_tiles = {}
            w2_tiles = {}

            def load_w1q(e, q):
                w1q = w1p.tile([P, KC * OQ * P], f32r, tag="w1q", name=f"w1q_{e}_{q}")
                nc.sync.dma_start(w1q[:], w1_d[e, q])
                w1_tiles[(e, q)] = w1q

            def load_w2q(e, q):
                w2q = w2p.tile([P, OQ * H], f32r, tag="w2q", name=f"w2q_{e}_{q}")
                nc.gpsimd.dma_start(w2q[:], w2_d[e, q])
                w2_tiles[(e, q)] = w2q

            load_w1q(0, 0)
            b1n_sb = const.tile([E, I], f32)
            nc.gpsimd.dma_start(b1n_sb[:], b1_d)
            xtr = const.tile([P, KC * T], f32r)
            nc.gpsimd.dma_start(xtr[:], xtr_d)
            load_w2q(0, 0)
            gwT = const.tile([E, T], f32)

            # ---- derive per-partition layouts of gate_w and b1 via PE ----
            # transpose([7, 128] block) -> psum [128, 7] -> SBUF
            ident7 = ident[:E, :E]
            gw_sb = const.tile([P, KC * E], f32)
            for k in range(KC):
                tpg = pwork.tile([P, E], f32, tag="pw")
                nc.tensor.matmul(
                    tpg[:], gwn_sb[:, k * P : (k + 1) * P], ident7, is_transpose=True
                )
                nc.vector.tensor_copy(gw_sb[:, k * E : (k + 1) * E], tpg[:])
            b1_sb = const.tile([P, IC * E], f32)
            for i in range(IC):
                tpb = pwork.tile([P, E], f32, tag="pw")
                nc.tensor.matmul(
                    tpb[:], b1n_sb[:, i * P : (i + 1) * P], ident7, is_transpose=True
                )
                nc.vector.tensor_copy(b1_sb[:, i * E : (i + 1) * E], tpb[:])

            # ---- gating: logits -> top-2 masked softmax (token-major) ----
            gwt_tiles = []
            for ti in range(NT):
                tsl = slice(ti * P, (ti + 1) * P)
                lg_ps = pwork.tile([P, E], f32, tag="pw")
                for k in range(KC):
                    nc.tensor.matmul(
                        lg_ps[:],
                        xt[:, k * T + ti * P : k * T + (ti + 1) * P],
                        gw_sb[:, k * E : (k + 1) * E],
                        start=(k == 0),
                        stop=False,
                    )
                nc.tensor.matmul(lg_ps[:], ones_row[:], gb_sb[:], start=False, stop=True)

                lg = gat.tile([P, E], f32, tag="lg")
                nc.vector.tensor_copy(lg[:], lg_ps[:])
                m1 = gat.tile([P, 1], f32, tag="m1")
                nc.vector.reduce_max(m1[:], lg[:], axis=AX.X)
                nm1 = gat.tile([P, 1], f32, tag="nm1")
                nc.vector.tensor_scalar_mul(nm1[:], m1[:], -1.0)
                eq = gat.tile([P, E], f32, tag="eq")
                nc.vector.tensor_scalar(eq[:], lg[:], m1[:], None, op0=OP.is_equal)
                masked = gat.tile([P, E], f32, tag="masked")
                nc.vector.scalar_tensor_tensor(
                    masked[:], eq[:], -1e30, lg[:], op0=OP.mult, op1=OP.add
                )
                m2 = gat.tile([P, 1], f32, tag="m2")
                nc.vector.reduce_max(m2[:], masked[:], axis=AX.X)
                ge = gat.tile([P, E], f32, tag="ge")
                nc.vector.tensor_scalar(ge[:], lg[:], m2[:], None, op0=OP.is_ge)
                ex = gat.tile([P, E], f32, tag="ex")
                nc.scalar.activation(ex[:], lg[:], AF.Exp, bias=nm1[:])
                ek = gat.tile([P, E], f32, tag="ek")
                nc.vector.tensor_tensor(ek[:], ex[:], ge[:], op=OP.mult)
                z = gat.tile([P, 1], f32, tag="z")
                nc.vector.reduce_sum(z[:], ek[:], axis=AX.X)
                rz = gat.tile([P, 1], f32, tag="rz")
                nc.vector.reciprocal(rz[:], z[:])
                gwt = gat.tile([P, E], f32, tag="gwt", bufs=NT)
                nc.vector.tensor_scalar_mul(gwt[:], ek[:], rz[:])
                gwt_tiles.append((tsl, gwt))
                # transpose [P, E] -> [E, P] and place into gwT
                tp = pwork.tile([E, P], f32, tag="pw")
                nc.tensor.matmul(tp[:], gwt[:], ident[:], is_transpose=True)
                nc.vector.tensor_copy(gwT[:, tsl], tp[:])
            for tsl, gwt in gwt_tiles:
                nc.scalar.dma_start(gw_d[tsl, :], gwt[:])

            # ---- init out accumulators with sum_e gw[e]*b2[e] (exact fp32) ----
            acc = []
            for j in range(KC):
                a = pacc.tile([P, T], f32, tag=f"acc{j}")
                nc.tensor.matmul(
                    a[:],
                    b2_sb[:, j * P : (j + 1) * P],
                    gwT[:],
                    start=True,
                    stop=False,
                )
                acc.append(a)

            # ---- expert loop ----
            for e in range(E):
                # broadcast gw row -> [P, T] (exact fp32: 1.0 * gw)
                bc_ps = pwork.tile([P, T], f32, tag="pw")
                nc.tensor.matmul(
                    bc_ps[:], sel[:, e * P : (e + 1) * P], gwT[:], start=True, stop=True
                )
                gwb = gwbp.tile([P, T], f32, tag="gwb")
                nc.vector.tensor_copy(gwb[:], bc_ps[:])

                for q in range(NQ):
                    if (e, q) not in w1_tiles:
                        load_w1q(e, q)
                    if (e, q) not in w2_tiles:
                        load_w2q(e, q)
                    w1q = w1_tiles.pop((e, q))
                    w2q = w2_tiles.pop((e, q))
                    for c in range(OQ):
                        i = q * OQ + c
                        hp_ps = pwork.tile([P, T], f32, tag="pw")
                        for k in range(KC):
                            nc.tensor.matmul(
                                hp_ps[:],
                                w1q[:, k * OQ * P + c * P : k * OQ * P + (c + 1) * P],
                                xtr[:, k * T : (k + 1) * T],
                                start=(k == 0),
                                stop=(k == KC - 1),
                            )
                        h_sb = hp.tile([P, T], f32, tag="h")
                        nc.scalar.activation(
                            h_sb[:],
                            hp_ps[:],
                            AF.Gelu,
                            bias=b1_sb[:, i * E + e : i * E + e + 1],
                        )
                        hs = hsp.tile([P, T], f32r, tag="hs")
                        nc.vector.tensor_tensor(hs[:], h_sb[:], gwb[:], op=OP.mult)
                        last = e == E - 1 and i == IC - 1
                        for j in range(KC):
                            nc.tensor.matmul(
                                acc[j][:],
                                w2q[:, c * H + j * P : c * H + (j + 1) * P],
                                hs[:],
                                start=False,
                                stop=last,
                            )

            # ---- drain accumulators ----
            for j in range(KC):
                osb = outp.tile([P, T], f32, tag="osb")
                nc.scalar.copy(osb[:], acc[j][:])
                nc.sync.dma_start(out_d[j], osb[:])

    nc.compile()
    return nc


def _get_program():
    if "nc" not in _CACHE:
        _CACHE["nc"] = _build_program()
    return _CACHE["nc"]


def _prep_shared(gate_w, gate_b, w1, b1, w2, b2):
    f = np.float32
    gate_w_r = np.ascontiguousarray(gate_w.T, dtype=f)
    gate_b_r = np.ascontiguousarray(gate_b.reshape(1, E), dtype=f)
    w1_r = np.ascontiguousarray(
        w1.reshape(E, KC, P, NQ, OQ * P).transpose(0, 3, 2, 1, 4), dtype=f
    ).reshape(E, NQ, P, KC * OQ * P)
    b1_r = np.ascontiguousarray(b1, dtype=f)
    w2_r = np.ascontiguousarray(
        w2.reshape(E, NQ, OQ, P, H).transpose(0, 1, 3, 2, 4), dtype=f
    ).reshape(E, NQ, P, OQ * H)
    b2_r = np.ascontiguousarray(b2, dtype=f)
    return {
        "gate_w_r": gate_w_r,
        "gate_b_r": gate_b_r,
        "w1_r": w1_r,
        "b1_r": b1_r,
        "w2_r": w2_r,
        "b2_r": b2_r,
        "sel_r": np.ascontiguousarray(
            np.broadcast_to(np.eye(E, dtype=f)[:, :, None], (E, E, P))
        ).reshape(E, E * P),
    }


def _make_in_maps(x, gate_w, gate_b, w1, b1, w2, b2):
    shared = _prep_shared(gate_w, gate_b, w1, b1, w2, b2)
    in_maps = []
    for c in range(B):
        xt = np.ascontiguousarray(
            np.asarray(x[c], dtype=np.float32).T.reshape(KC, P, T).transpose(1, 0, 2)
        ).reshape(P, KC * T)
        m = dict(shared)
        m["xt"] = xt
        m["xt_r"] = xt
        in_maps.append(m)
    return in_maps


def run_on_hw(in_maps, trace=False):
    from concourse.bass_interp import get_hw_module
    from concourse.bass_utils import run_bass_kernel_spmd

    nc = _get_program()
    old_m = nc.m
    nc.m = get_hw_module(nc.m)
    try:
        return run_bass_kernel_spmd(
            nc, in_maps, core_ids=list(range(B)), trace=trace
        )
    finally:
        nc.m = old_m


def _assemble(results):
    out = np.empty((B, S, H), dtype=np.float32)
    gws = np.empty((B, S, E), dtype=np.float32)
    for c in range(B):
        out_fm = results[c]["out_fm"]  # [KC, P, T]
        out[c] = out_fm.transpose(2, 0, 1).reshape(T, H)
        gws[c] = results[c]["gw_out"]
    return out, gws


def kernel(x, gate_w, gate_b, w1, b1, w2, b2):
    in_maps = _make_in_maps(x, gate_w, gate_b, w1, b1, w2, b2)
    res = run_on_hw(in_maps, trace=False)
    return _assemble(res.results)
